# revision 8
# baseline (speedup 1.0000x reference)
"""Trainium2 Bass kernel v2 for nn_MicroExpert (sparse_attention).

Reference model (B=2, T=2048, D=512, H=8, HD=64):
  v_in = conv1d(x, k=3, pad=1); MHA(q=x, k=x, v=v_in) with banded mask
  |i-j| <= 256; h = LN(x + attn); out = LN(h + FFN(h)).

Sharding: data-parallel over (batch, 512-token chunk) -> 8 independent
cores, no collectives.  Each core recomputes the K/V halo (+-256 tokens,
zero-padded at sequence edges; pad keys are neutralized exactly via a
denominator correction `padcnt`).

v2 changes vs the baseline:
  - Scalar engine does ONLY exp (16 wide [128,1280] activations, one per
    (query-tile, head-pair)) plus 2 ln/exp pairs for the LN rstd
    (rstd = exp(-0.5*ln(var+eps)) -- same activation table set as exp, so
    zero ACT_TABLE_LOAD churn).  All copies/squares moved to DVE.
  - LayerNorm runs entirely in the transposed [feature, token] layout:
    per-token sums via ones-column matmuls (partition reduction), mean/
    rstd broadcast via ones-row matmuls.  No PE transposes at all.
  - Scores for a head pair go into one [128,1280] PSUM tile (A|B
    stacked), exp'd in a single ACTIVATE; score matmuls for the two heads
    alternate lhsT base partitions 0/64 so the PE row-tiles them
    concurrently.
  - Softmax denominator: ones-row in v (row 64 of each [128,65] v tile)
    accumulates sum(exp) during ctx; per-pair normalization uses a
    ones-row broadcast matmul instead of the DMA round-trip.
  - Weight/activation DMA loads are emitted in first-needed order
    (xt, wk first) so Kproj can start early.
"""

import sys

import numpy as np

sys.path.insert(0, "/opt/trn_rl_repo")

import concourse.bass as bass
import concourse.mybir as mybir
import concourse.tile as tile
from concourse import bacc
from concourse.bass_utils import run_bass_kernel_spmd

BF16 = mybir.dt.bfloat16
F32 = mybir.dt.float32
AF = mybir.ActivationFunctionType
OP = mybir.AluOpType

B, T, D, H, HD = 2, 2048, 512, 8, 64
S = 512          # tokens per core
KV = 1024        # extended kv tokens per core (S + 2*256)
XE = 1026        # x_ext width (KV + 2 for conv halo)
F = 1024         # FFN hidden
EPS = 1e-5
N_CORES = 8

_cached = {}


def _build_program():
    nc = bacc.Bacc("TRN2", target_bir_lowering=False, debug=False)

    def din(name, shape, dt):
        return nc.dram_tensor(name, shape, dt, kind="ExternalInput").ap()

    # all inputs are pre-layouted [128, N] SBUF images (host does the packing)
    xt_d = din("xt", [128, 4 * XE], BF16)
    wk_d = din("wk", [128, 2048], BF16)
    wq_d = din("wq", [128, 2048], BF16)
    wu_d = din("wu", [128, 6144], BF16)
    wo_d = din("wo", [128, 2048], BF16)
    w1_d = din("w1", [128, 4096], BF16)
    w2_d = din("w2", [128, 4096], BF16)
    b1_d = din("b1", [128, 8], F32)
    b2_d = din("b2", [128, 4], F32)
    mask_d = din("mask01", [128, 256], BF16)          # [tril | triu] 0/1
    nw1s_d = din("nw1s", [1, 1024], BF16)             # -sum_d w1[f, d]
    padcnt_d = din("padcnt", [8, 512], F32)           # [h, q] (rows identical)
    padcntb_d = din("padcntb", [1, 512], BF16)        # exact (counts <= 256)

    out_d = nc.dram_tensor("out", [128, 2048], BF16, kind="ExternalOutput").ap()

    with tile.TileContext(nc) as tc:
        from contextlib import ExitStack

        with ExitStack() as ctx:
            const = ctx.enter_context(tc.tile_pool(name="const", bufs=1))

            # ---- SBUF residents -----------------------------------------
            xt_sb = const.tile([128, 4 * XE], BF16)
            wk_sb = const.tile([128, 2048], BF16)
            wq_sb = const.tile([128, 2048], BF16)
            wu_sb = const.tile([128, 6144], BF16)
            wo_sb = const.tile([128, 2048], BF16)
            w1_sb = const.tile([128, 4096], BF16)
            w2_sb = const.tile([128, 4096], BF16)
            b1_sb = const.tile([128, 8], F32)
            b2_sb = const.tile([128, 4], F32)
            mask_sb = const.tile([128, 256], BF16)
            nw1s_sb = const.tile([1, 1024], BF16)
            padcnt_sb = const.tile([8, 512], F32)
            padcntb_sb = const.tile([1, 512], BF16)
            negones_sb = const.tile([1, 64], BF16)
            ones_sb = const.tile([128, 128], BF16)
            eps_sb = const.tile([1, 1], F32)
            epsd2_sb = const.tile([1, 1], F32)

            kt_sb = const.tile([128, 4 * KV], BF16)    # [oc-block][kv]
            q_sb = const.tile([128, 4 * S], BF16)      # [oc-block][tok]
            v_sb = const.tile([128, 8 * 520], BF16)    # [kvtile][(v_h|1) x 8]
            ctx_sb = const.tile([128, 4 * 512], BF16)  # [pair-block][q]
            r1t_sb = const.tile([128, 4 * 512], BF16)  # [dc-block][tok]
            hnt_sb = const.tile([128, 4 * 512], BF16)  # LN1 out [dc-block][tok]
            m1_sb = const.tile([128, 8 * 512], BF16)   # [fc-block][tok]
            r2t_sb = const.tile([128, 4 * 512], BF16)  # [dc-block][tok]
            out_sb = const.tile([128, 4 * 512], BF16)
            rows_sb = const.tile([1, 4 * 1024 + 32], F32)  # stat scratch rows
            rowsb_sb = const.tile([1, 2 * 1024 + 512], BF16)  # bcast-rhs rows
            onesf_sb = const.tile([1, 1], F32)
            dcol_sb = const.tile([128, 1], BF16)

            # ---- loads: one DMA per tensor, first-needed first ----------
            # (dma_start instructions issue serially at ~650ns each on the
            # Sync HWDGE ring and transfers drain FIFO, so order = priority)
            nc.sync.dma_start(wk_sb[:], wk_d[:])
            for c in range(4):
                nc.sync.dma_start(
                    xt_sb[:, XE * c: XE * c + XE], xt_d[:, XE * c: XE * c + XE])
            nc.sync.dma_start(wq_sb[:], wq_d[:])
            nc.sync.dma_start(wu_sb[:], wu_d[:])
            nc.sync.dma_start(mask_sb[:], mask_d[:])
            nc.sync.dma_start(nw1s_sb[:], nw1s_d[:])
            nc.sync.dma_start(padcnt_sb[:], padcnt_d[:])
            nc.sync.dma_start(padcntb_sb[:], padcntb_d[:])
            nc.sync.dma_start(wo_sb[:], wo_d[:])
            nc.sync.dma_start(b1_sb[:], b1_d[:])
            nc.sync.dma_start(b2_sb[:], b2_d[:])
            nc.sync.dma_start(w1_sb[:], w1_d[:])
            nc.sync.dma_start(w2_sb[:], w2_d[:])
            nc.gpsimd.memset(ones_sb[:], 1.0)
            nc.gpsimd.memset(negones_sb[:], -1.0)
            nc.gpsimd.memset(v_sb[:], 1.0)
            nc.gpsimd.memset(eps_sb[:], float(EPS))
            nc.gpsimd.memset(epsd2_sb[:], float(EPS * D * D))
            nc.gpsimd.memset(onesf_sb[:], 1.0)
            nc.gpsimd.memset(dcol_sb[:], float(D))

            def hb(scratch_ps, dep_row, f32=True):
                # HAM-warming pokes measured neutral-to-negative; disabled
                return

            xt_v = xt_sb[:].rearrange("p (c w) -> p c w", c=4)

            # ---- K/Q projections ---------------------------------------
            with tc.tile_pool(name="pp", bufs=2, space="PSUM") as pp_pool:
                for oc in range(4):
                    for half in range(2):
                        pp = pp_pool.tile([128, 512], F32, tag="pp")
                        for kc in range(4):
                            nc.tensor.matmul(
                                pp[:],
                                wk_sb[:, 128 * (4 * kc + oc):128 * (4 * kc + oc) + 128],
                                xt_sb[:, XE * kc + 1 + 512 * half: XE * kc + 1 + 512 * half + 512],
                                start=(kc == 0), stop=(kc == 3),
                            )
                        nc.scalar.copy(
                            kt_sb[:, KV * oc + 512 * half: KV * oc + 512 * half + 512],
                            pp[:],
                        )
                for oc in range(4):
                    pp = pp_pool.tile([128, 512], F32, tag="pp")
                    for kc in range(4):
                        nc.tensor.matmul(
                            pp[:],
                            wq_sb[:, 128 * (4 * kc + oc):128 * (4 * kc + oc) + 128],
                            xt_sb[:, XE * kc + 257: XE * kc + 257 + 512],
                            start=(kc == 0), stop=(kc == 3),
                        )
                    nc.scalar.copy(q_sb[:, 512 * oc: 512 * oc + 512], pp[:])

            # ---- A1: scores (+exp) interleaved with Vproj ---------------
            # chunk (p, qt): [128,1280] psum: head 2p scores cols 0:640
            # (5 kt tiles of 128), head 2p+1 at 640:1280.
            ex_tiles = {}

            def emit_scores(sc_pool, ex_pool, p, qt):
                sc = sc_pool.tile([128, 1280], F32, tag="sc")
                for r in range(5):
                    for par in range(2):
                        hp = 64 * par
                        nc.tensor.matmul(
                            sc[:, 640 * par + 128 * r: 640 * par + 128 * r + 128],
                            kt_sb[hp:hp + 64, KV * p + 128 * (qt + r): KV * p + 128 * (qt + r) + 128],
                            q_sb[hp:hp + 64, 512 * p + 128 * qt: 512 * p + 128 * qt + 128],
                            start=True, stop=True,
                        )
                ex = ex_pool.tile([128, 1280], BF16, tag="ex")
                nc.scalar.activation(
                    ex[:], sc[:], AF.Exp, scale=float(1.0 / np.sqrt(HD)),
                )
                # band mask on relative tiles 0 and 4 of each head
                exr = ex[:].rearrange("p (a b) -> p a b", a=10)
                mv = mask_sb[:].rearrange("p (n w) -> p n w", n=2)
                nc.vector.tensor_mul(exr[:, 0:5:4, :], exr[:, 0:5:4, :], mv)
                nc.vector.tensor_mul(exr[:, 5:10:4, :], exr[:, 5:10:4, :], mv)
                ex_tiles[(p, qt)] = ex

            def emit_vproj(v_pool, tt):
                pp = v_pool.tile([128, 512], F32, tag="vp")
                n = 0
                for tap in range(3):
                    for dc in range(4):
                        nc.tensor.matmul(
                            pp[:],
                            xt_sb[:, XE * dc + 128 * tt + tap: XE * dc + 128 * tt + tap + 128],
                            wu_sb[:, 512 * (4 * tap + dc): 512 * (4 * tap + dc) + 512],
                            start=(n == 0), stop=(n == 11),
                        )
                        n += 1
                vv = v_sb[:, 520 * tt: 520 * tt + 520].rearrange(
                    "p (h w) -> p h w", h=8
                )
                nc.vector.tensor_copy(
                    vv[:, :, 0:64], pp[:].rearrange("p (h w) -> p h w", h=8)
                )

            with ExitStack() as aouter:
                ex_pool = aouter.enter_context(tc.tile_pool(name="exsb", bufs=16))

                with ExitStack() as a1:
                    sc_pool = a1.enter_context(
                        tc.tile_pool(name="scps", bufs=2, space="PSUM"))
                    v_pool = a1.enter_context(
                        tc.tile_pool(name="vps", bufs=2, space="PSUM"))

                    chunks = [(p, qt) for p in range(4) for qt in range(4)]
                    emit_scores(sc_pool, ex_pool, *chunks[0])
                    emit_scores(sc_pool, ex_pool, *chunks[1])
                    ci = 2
                    for tt in range(8):
                        emit_vproj(v_pool, tt)
                        for _ in range(2):
                            if ci < 16:
                                emit_scores(sc_pool, ex_pool, *chunks[ci])
                                ci += 1

                # dummy sqrt: pulls the sqrt ACT_TABLE_LOAD off the LN1
                # critical path (loads while the PE runs ctx matmuls)
                nc.scalar.activation(
                    rows_sb[0:1, 4096:4112], ex_tiles[(3, 3)][0:1, 0:16],
                    AF.Sqrt)

                # ---- A2: ctx accumulation + per-pair normalize ----------
                with ExitStack() as a2:
                    cx_pool = a2.enter_context(
                        tc.tile_pool(name="cxps", bufs=3, space="PSUM"))
                    bc_pool = a2.enter_context(
                        tc.tile_pool(name="bcps", bufs=2, space="PSUM"))
                    dn_pool = a2.enter_context(tc.tile_pool(name="dnsb", bufs=3))

                    def norm_front(p, cps):
                        # den rows (psum f32 -> sbuf bf16 casts, on Scalar)
                        den = dn_pool.tile([1, 1024], BF16, tag="den")
                        nc.scalar.copy(den[0:1, 0:512], cps[0][64:65, :])
                        nc.scalar.copy(den[0:1, 512:1024], cps[1][64:65, :])
                        return den

                    def norm_back(p, cps, den):
                        # bcast (den - padcnt) via two accumulating ones-row
                        # matmuls per half, then wide [128,512] DVE ops
                        bc = bc_pool.tile([128, 512], F32, tag="bc")
                        for par in range(2):
                            nc.tensor.matmul(
                                bc[64 * par:64 * par + 64, :],
                                ones_sb[0:1, 0:64],
                                den[0:1, 512 * par: 512 * par + 512],
                                start=True, stop=False,
                                skip_group_check=True,
                                tile_position=(0, 64 * par),
                            )
                            nc.tensor.matmul(
                                bc[64 * par:64 * par + 64, :],
                                negones_sb[0:1, 0:64],
                                padcntb_sb[0:1, :],
                                start=False, stop=True,
                                skip_group_check=True,
                                tile_position=(0, 64 * par),
                            )
                        recf = dn_pool.tile([128, 512], F32, tag="recf")
                        nc.vector.reciprocal_approx_fast(recf[:], bc[:])
                        hb(bc, recf[0:1, 0:64])
                        for par in range(2):
                            nc.vector.tensor_mul(
                                ctx_sb[64 * par:64 * par + 64, 512 * p: 512 * p + 512],
                                cps[par][0:64, :],
                                recf[64 * par:64 * par + 64, :],
                            )
                        hb(bc, ctx_sb[0:1, 512 * p: 512 * p + 64], f32=False)

                    pending = None
                    for p in range(4):
                        cxA = cx_pool.tile([65, 512], F32, tag="cxA")
                        cxB = cx_pool.tile([65, 512], F32, tag="cxB")
                        cps = (cxA, cxB)
                        for qt in range(4):
                            ex = ex_tiles[(p, qt)]
                            for r in range(5):
                                k = qt + r
                                for par in range(2):
                                    nc.tensor.matmul(
                                        cps[par][0:65, 128 * qt: 128 * qt + 128],
                                        v_sb[:, 520 * k + 65 * (2 * p + par): 520 * k + 65 * (2 * p + par) + 65],
                                        ex[:, 640 * par + 128 * r: 640 * par + 128 * r + 128],
                                        start=(qt == 0 and r == 0),
                                        stop=(qt == 3 and r == 4),
                                        skip_group_check=True,
                                    )
                        rdenb = norm_front(p, cps)
                        if pending is not None:
                            norm_back(*pending)
                        pending = (p, cps, rdenb)
                    norm_back(*pending)

                    # out_proj inside the A2 scope, one rotating bc-pool
                    # bank per oc group: starts as earlier pairs' bc tiles
                    # retire instead of waiting for the full pool handoff
                    for oc in range(4):
                        atp = bc_pool.tile([128, 512], F32, tag="bc",
                                           name=f"atp{oc}")
                        for pc in range(4):
                            nc.tensor.matmul(
                                atp[:],
                                wo_sb[:, 128 * (4 * pc + oc): 128 * (4 * pc + oc) + 128],
                                ctx_sb[:, 512 * pc: 512 * pc + 512],
                                start=(pc == 0), stop=(pc == 3),
                            )
                        nc.vector.tensor_add(
                            r1t_sb[:, 512 * oc: 512 * oc + 512], atp[:],
                            xt_v[:, oc, 257:769],
                        )

            # ---- out_proj + residual + LN1 ------------------------------
            def ln_rows(stats_ps, base, bc):
                # rstd = D / sqrt(D*ssq - s^2 + eps*D^2); rowsb gets
                # (mean | rstd) bf16
                s_ps = stats_ps[0:1, 0:512]
                ssq_ps = stats_ps[0:1, 512:1024]
                m2 = rows_sb[0:1, 2048 + base: 2048 + base + 512]
                nc.scalar.activation(m2, s_ps, AF.Square)          # s^2
                t = rows_sb[0:1, 2048 + base + 512: 2048 + base + 1024]
                nc.scalar.mul(rowsb_sb[0:1, base: base + 512], s_ps, 1.0 / D)
                nc.vector.tensor_sub(t, ssq_ps, m2)                # D^2 * var
                nc.scalar.activation(m2, t, AF.Sqrt,
                                     bias=epsd2_sb[0:1, 0:1])      # D*sigma
                nc.vector.reciprocal_approx_fast(t, m2)
                nc.scalar.mul(rowsb_sb[0:1, base + 512: base + 1024], t,
                              float(D))

            def ln_bcast(bc, base):
                for half in range(2):
                    nc.tensor.matmul(
                        bc[:, 512 * half: 512 * half + 512],
                        ones_sb[0:1, 0:128],
                        rowsb_sb[0:1, base + 512 * half: base + 512 * half + 512],
                        start=True, stop=True, skip_group_check=True,
                    )

            # ---- out_proj + LN1 + FFN1 (FFN1 GEMMs run on the raw
            # residual r1t; the LN mean folds in as a rank-1 PE
            # accumulation and rstd applies columnwise at drain time, so
            # the whole LN1 stats chain hides under the GEMMs) ----------
            with ExitStack() as o1:
                # stats and bc1 share one 2-bank slot sequentially (bc1 is
                # only written after the chain has consumed stats), so
                # f_pool below can hold 6 GEMM buffers
                st_pool = o1.enter_context(
                    tc.tile_pool(name="stps", bufs=1, space="PSUM"))
                sq_pool = o1.enter_context(tc.tile_pool(name="sqsb", bufs=2))
                tm_pool = o1.enter_context(tc.tile_pool(name="tmsb", bufs=4))

                stats = st_pool.tile([1, 1024], F32, tag="stlb")
                for dc in range(4):
                    sl = slice(512 * dc, 512 * dc + 512)
                    sq = sq_pool.tile([128, 512], BF16, tag="sq")
                    nc.scalar.activation(sq[:], r1t_sb[:, sl], AF.Square)
                    nc.tensor.matmul(
                        stats[0:1, 0:512], ones_sb[:, 0:1], r1t_sb[:, sl],
                        start=(dc == 0), stop=(dc == 3), skip_group_check=True,
                    )
                    nc.tensor.matmul(
                        stats[0:1, 512:1024], dcol_sb[:, 0:1], sq[:],
                        start=(dc == 0), stop=(dc == 3), skip_group_check=True,
                    )
                # atps banks released here -> f_pool can take 6
                f_pool = o1.enter_context(
                    tc.tile_pool(name="fps", bufs=6, space="PSUM"))

                phi = rowsb_sb[0:1, 2048: 2048 + 512]

                def ffn1_g(fc):
                    g = f_pool.tile([128, 512], F32, tag="f")
                    for dc in range(4):
                        nc.tensor.matmul(
                            g[:],
                            w1_sb[:, 128 * (8 * dc + fc): 128 * (8 * dc + fc) + 128],
                            r1t_sb[:, 512 * dc: 512 * dc + 512],
                            start=(dc == 0), stop=False, skip_group_check=True,
                        )
                    return g

                def ffn1_fin(fc, g):
                    # G += (-sum_d w1) (x) (mu*rstd); m1 = Relu(G*rstd + b1)
                    nc.tensor.matmul(
                        g[:], nw1s_sb[0:1, 128 * fc: 128 * fc + 128], phi,
                        start=False, stop=True, skip_group_check=True,
                    )
                    t1 = tm_pool.tile([128, 512], BF16, tag="t1")
                    nc.vector.tensor_mul(t1[:], g[:], bc1s[:, 512:1024])
                    nc.scalar.activation(
                        m1_sb[:, 512 * fc: 512 * fc + 512], t1[:],
                        AF.Relu, bias=b1_sb[:, fc:fc + 1],
                    )

                gq = []
                for fc in range(5):
                    gq.append((fc, ffn1_g(fc)))
                ln_rows(stats, 0, stats)
                nc.vector.tensor_mul(
                    phi, rowsb_sb[0:1, 0:512], rowsb_sb[0:1, 512:1024])
                bc1 = st_pool.tile([128, 1024], F32, tag="stlb", name="bc1t")
                ln_bcast(bc1, 0)
                bc1s = tm_pool.tile([128, 1024], BF16, tag="bcs")
                nc.vector.tensor_copy(bc1s[:], bc1[:])
                for fc in range(5, 8):
                    gq.append((fc, ffn1_g(fc)))
                    ffn1_fin(*gq.pop(0))
                while gq:
                    ffn1_fin(*gq.pop(0))
                # hnt (normalized h) still needed for the second residual
                tmps = []
                for dc in range(4):
                    sl = slice(512 * dc, 512 * dc + 512)
                    tmp = tm_pool.tile([128, 512], BF16, tag="tm")
                    nc.vector.tensor_sub(tmp[:], r1t_sb[:, sl], bc1s[:, 0:512])
                    tmps.append(tmp)
                for dc in range(4):
                    sl = slice(512 * dc, 512 * dc + 512)
                    nc.vector.tensor_mul(
                        hnt_sb[:, sl], tmps[dc][:], bc1s[:, 512:1024])

            # ---- FFN2 + residual + LN2 + store, pipelined over token
            # halves: the left half's LN2 chain + normalize + store run
            # while the PE computes the right half's GEMMs ---------------
            with ExitStack() as f1:
                # per-half stats and bc broadcast share a 2-bank slot
                # (bc is only written after the chain consumed the stats),
                # so f_pool gets 4 GEMM buffers instead of 2
                f_pool = f1.enter_context(
                    tc.tile_pool(name="fps2", bufs=4, space="PSUM"))
                lb_pool = f1.enter_context(
                    tc.tile_pool(name="lbps2", bufs=1, space="PSUM"))
                sq_pool = f1.enter_context(tc.tile_pool(name="sqsb2", bufs=2))
                tm_pool = f1.enter_context(tc.tile_pool(name="tmsb2", bufs=3))

                statsh = [lb_pool.tile([1, 1024], F32, tag=f"sl{h}",
                                       name=f"st2h{h}")
                          for h in range(2)]
                bch = {}

                def st2_mms(h, oc, sq):
                    sl = slice(512 * oc + 256 * h, 512 * oc + 256 * h + 256)
                    nc.tensor.matmul(
                        statsh[h][0:1, 0:256],
                        ones_sb[:, 0:1], r2t_sb[:, sl],
                        start=(oc == 0), stop=(oc == 3), skip_group_check=True,
                    )
                    nc.tensor.matmul(
                        statsh[h][0:1, 512:768],
                        dcol_sb[:, 0:1], sq[:],
                        start=(oc == 0), stop=(oc == 3), skip_group_check=True,
                    )

                def ln2_rows(h):
                    # rstd = D / sqrt(D*ssq - s^2 + eps*D^2), via row ops
                    base = 3072 + 512 * h
                    s_ps = statsh[h][0:1, 0:256]
                    ssq_ps = statsh[h][0:1, 512:768]
                    ob = 1024 + 512 * h
                    nc.scalar.mul(rowsb_sb[0:1, ob: ob + 256], s_ps, 1.0 / D)
                    m2 = rows_sb[0:1, base: base + 256]
                    nc.scalar.activation(m2, s_ps, AF.Square)
                    t = rows_sb[0:1, base + 256: base + 512]
                    nc.vector.tensor_sub(t, ssq_ps, m2)
                    nc.scalar.activation(m2, t, AF.Sqrt,
                                         bias=epsd2_sb[0:1, 0:1])
                    nc.vector.reciprocal_approx_fast(t, m2)
                    nc.scalar.mul(rowsb_sb[0:1, ob + 256: ob + 512], t,
                                  float(D))

                def ln2_bc(h):
                    ob = 1024 + 512 * h
                    bch[h] = lb_pool.tile([128, 512], F32, tag=f"sl{h}",
                                          name=f"bch{h}")
                    for half2 in range(2):
                        nc.tensor.matmul(
                            bch[h][:, 256 * half2: 256 * half2 + 256],
                            ones_sb[0:1, 0:128],
                            rowsb_sb[0:1, ob + 256 * half2: ob + 256 * half2 + 256],
                            start=True, stop=True, skip_group_check=True,
                        )

                def ln2_out(h):
                    r2v = r2t_sb[:].rearrange("p (dc w) -> p dc w", dc=8)
                    ov = out_sb[:].rearrange("p (g w) -> p g w", g=8)
                    muv = bch[h][:, 0:256].rearrange(
                        "p (g w) -> p g w", g=1).broadcast_to([128, 2, 256])
                    rsv = bch[h][:, 256:512].rearrange(
                        "p (g w) -> p g w", g=1).broadcast_to([128, 2, 256])
                    tvs = []
                    for dp in range(2):
                        # dc pair (2*dp, 2*dp+1), half h columns
                        tmp = tm_pool.tile([128, 512], BF16, tag="tmo")
                        tv = tmp[:].rearrange("p (g w) -> p g w", g=2)
                        nc.vector.tensor_sub(
                            tv, r2v[:, 4 * dp + h:4 * dp + h + 3:2, :], muv)
                        tvs.append(tv)
                    for dp in range(2):
                        nc.vector.tensor_mul(
                            ov[:, 4 * h + 2 * dp: 4 * h + 2 * dp + 2, :],
                            tvs[dp], rsv)
                    nc.sync.dma_start(
                        out_d[:, 1024 * h: 1024 * h + 1024],
                        out_sb[:, 1024 * h: 1024 * h + 1024])

                def gemms(h, mid=None):
                    pend = None
                    for oc in range(4):
                        sl = slice(512 * oc + 256 * h, 512 * oc + 256 * h + 256)
                        fps = f_pool.tile([128, 512], F32, tag="f")
                        for fc in range(8):
                            nc.tensor.matmul(
                                fps[:, 0:256],
                                w2_sb[:, 128 * (4 * fc + oc): 128 * (4 * fc + oc) + 128],
                                m1_sb[:, 512 * fc + 256 * h: 512 * fc + 256 * h + 256],
                                start=(fc == 0), stop=(fc == 7),
                            )
                        if pend is not None:
                            st2_mms(h, *pend)
                        if oc == 2 and mid is not None:
                            mid()
                        f2 = tm_pool.tile([128, 256], BF16, tag="f2")
                        nc.scalar.activation(
                            f2[:], fps[:, 0:256], AF.Identity,
                            bias=b2_sb[:, oc:oc + 1])
                        nc.vector.tensor_add(r2t_sb[:, sl], f2[:], hnt_sb[:, sl])
                        sq = sq_pool.tile([128, 256], BF16, tag="sq2")
                        nc.scalar.activation(sq[:], r2t_sb[:, sl], AF.Square)
                        pend = (oc, sq)
                    st2_mms(h, *pend)

                gemms(0)
                ln2_rows(0)

                def mid0():
                    ln2_bc(0)
                    ln2_out(0)

                gemms(1, mid=mid0)
                ln2_rows(1)
                ln2_bc(1)
                ln2_out(1)

    nc.compile()
    return nc


def _prep_host(inputs):
    x = np.asarray(inputs["x"], np.float32)
    conv_w = np.asarray(inputs["conv_w"], np.float32)
    conv_b = np.asarray(inputs["conv_b"], np.float32)
    in_w = np.asarray(inputs["in_proj_w"], np.float32)
    in_b = np.asarray(inputs["in_proj_b"], np.float32)
    out_w = np.asarray(inputs["out_proj_w"], np.float32)
    out_b = np.asarray(inputs["out_proj_b"], np.float32)
    w1 = np.asarray(inputs["w1"], np.float32)
    b1 = np.asarray(inputs["b1"], np.float32)
    w2 = np.asarray(inputs["w2"], np.float32)
    b2 = np.asarray(inputs["b2"], np.float32)
    g1 = np.asarray(inputs["ln1_g"], np.float32)
    bb1 = np.asarray(inputs["ln1_b"], np.float32)
    g2 = np.asarray(inputs["ln2_g"], np.float32)
    bb2 = np.asarray(inputs["ln2_b"], np.float32)

    for nm, v in (("conv_b", conv_b), ("in_proj_b", in_b), ("out_proj_b", out_b)):
        if np.any(v != 0):
            raise NotImplementedError(f"nonzero {nm} unsupported")
    if np.any(g1 != 1) or np.any(bb1 != 0) or np.any(g2 != 1) or np.any(bb2 != 0):
        raise NotImplementedError("nontrivial layernorm affine unsupported")

    Wq, Wk, Wv = in_w[:D], in_w[D:2 * D], in_w[2 * D:]
    U = [(Wv @ conv_w[:, :, d]) for d in range(3)]  # v[t] = sum U_d @ x[t+d-1]

    def img(stack):  # [n, 128, w] slices -> [128, n*w] SBUF image
        a = np.asarray(stack, np.float32)
        return np.ascontiguousarray(a.transpose(1, 0, 2).reshape(128, -1))

    def slc16(W):  # W used as out = W @ x  -> lhsT slices of W.T, oc-major
        WT = np.ascontiguousarray(W.T)
        return img([
            WT[128 * kc:128 * kc + 128, 128 * oc:128 * oc + 128]
            for kc in range(4) for oc in range(4)
        ])

    wk_a = slc16(Wk)
    wq_a = slc16(Wq)
    wo_a = slc16(out_w)
    wu_a = img([
        np.ascontiguousarray(U[tap].T)[128 * dc:128 * dc + 128, :]
        for tap in range(3) for dc in range(4)
    ])
    w1_a = img([
        np.ascontiguousarray(w1.T)[128 * dc:128 * dc + 128, 128 * fc:128 * fc + 128]
        for dc in range(4) for fc in range(8)
    ])
    w2_a = img([
        np.ascontiguousarray(w2.T)[128 * fc:128 * fc + 128, 128 * oc:128 * oc + 128]
        for fc in range(8) for oc in range(4)
    ])
    b1_a = np.ascontiguousarray(b1.reshape(8, 128).T)
    b2_a = np.ascontiguousarray(b2.reshape(4, 128).T)

    r = np.arange(128)
    m_lo = (r[:, None] >= r[None, :]).astype(np.float32)   # block 0: keep k>=q
    mask01 = np.concatenate([m_lo, m_lo.T], axis=1)

    def bf(a):
        import ml_dtypes
        return np.asarray(a, dtype=ml_dtypes.bfloat16)

    common = {
        "wk": bf(wk_a), "wq": bf(wq_a), "wu": bf(wu_a), "wo": bf(wo_a),
        "w1": bf(w1_a), "w2": bf(w2_a), "b1": b1_a.astype(np.float32),
        "b2": b2_a.astype(np.float32), "mask01": bf(mask01),
        "nw1s": bf(-w1.sum(axis=1).reshape(1, 1024)),
    }

    in_maps = []
    for c in range(N_CORES):
        b, j = divmod(c, 4)
        s = 512 * j
        xe = np.zeros((XE, D), np.float32)
        lo, hi = max(0, s - 257), min(T, s + 769)
        xe[lo - (s - 257): hi - (s - 257)] = x[b, lo:hi]
        xt = xe.T.reshape(4, 128, XE).transpose(1, 0, 2).reshape(128, 4 * XE)
        xt = np.ascontiguousarray(xt)

        # padcnt[qt, r]: in-band-kept pad keys
        key = (s - 256 + 128 * np.arange(4)[:, None, None]
               + np.arange(640)[None, None, :])          # [qt,1,640]
        pad = (key < 0) | (key >= T)
        cc, rr = np.arange(640)[None, None, :], r[None, :, None]
        kept = ((cc >= 128) & (cc < 512)) | ((cc < 128) & (cc >= rr)) \
            | ((cc >= 512) & (cc - 512 <= rr))
        pc = (pad & kept).sum(axis=2).astype(np.float32)  # [4, 128]
        padcnt = np.broadcast_to(pc.reshape(1, 512), (8, 512)).copy()

        m = dict(common)
        m["xt"] = bf(xt)
        m["padcnt"] = padcnt
        m["padcntb"] = bf(padcnt[0:1])
        in_maps.append(m)
    return in_maps


def kernel(**inputs) -> np.ndarray:
    if "nc" not in _cached:
        _cached["nc"] = _build_program()
    nc = _cached["nc"]
    in_maps = _prep_host(inputs)
    res = run_bass_kernel_spmd(nc, in_maps, core_ids=list(range(N_CORES)))
    out = np.empty((B, T, D), np.float32)
    for c in range(N_CORES):
        b, j = divmod(c, 4)
        o = res.results[c]["out"].astype(np.float32).reshape(128, 2, 4, 256)
        out[b, 512 * j: 512 * j + 512] = \
            o.transpose(1, 3, 2, 0).reshape(512, 512)
    return out
